# revision 17
# baseline (speedup 1.0000x reference)
"""Tensor-parallel attention kernel for 8 Trainium2 NeuronCores.

Reference computation (S=2048, B=2, H=2048, NH=16 heads, HD=128):
    q = x @ wq.T ; k = x @ wk.T ; v = x @ wv.T          (x: [S, B, H])
    per (b, head): out = softmax(q k^T / sqrt(HD)) v
    return concat_heads(out) @ wo.T                      ([S, B, H])

Sharding: tensor-parallel over heads. Core r owns heads {2r, 2r+1}:
column-parallel wq/wk/wv shards, row-parallel wo shard, ReduceScatter(add)
over the partial outputs; the host concatenates the 8 scatter slices.

On-core dataflow (bf16 matmuls, f32 accumulation; tokens b-major t = b*S+s):
  phase 1 (per batch): qT/kT [256 feat, 2048 tok] = wT.T @ xT;
                       v [2048 tok, 256 feat] natural
  phase 2 (per b,h,q-tile): scoresT [j, q] blocks -> one wide exp on
      ScalarE -> pT bf16; row sums via ones-matmul (gives broadcast rows);
      PV matmul -> oT [feat, tok]; normalize by 1/sums on VectorE
  phase 3 (per 1024-token chunk): partial[chunk] = oT-slices @ woT ->
      ReduceScatter(add) issued per chunk so it overlaps later compute
"""

import numpy as np

S, B, H = 2048, 2, 2048
NH, HD = 16, 128
N_CORES = 8
HPC = NH // N_CORES          # heads per core (2)
FPC = HPC * HD               # features per core (256)
NT = S * B                   # tokens (4096)
SCALE = HD ** -0.5
KT = H // 128                # contraction tiles in phase 1 (16)
NB = 512                     # token block width in phase 1
QT = 512                     # q-tile width in phase 2
EXPW = 2048                  # exp batch width (4 key-blocks per ACT op)
RS_CHUNKS = 4
CHUNK = NT // RS_CHUNKS      # tokens per chunk (1024)
SLICE = CHUNK // N_CORES     # rows a core receives per chunk (128)


def _build():
    import concourse.mybir as mybir
    import concourse.tile as tile
    from concourse import bacc

    F32 = mybir.dt.float32
    BF16 = mybir.dt.bfloat16
    Exp = mybir.ActivationFunctionType.Exp

    nc = bacc.Bacc(None, target_bir_lowering=False, num_devices=N_CORES)

    xT = nc.dram_tensor("xT", [H, NT], BF16, kind="ExternalInput")
    wqT = nc.dram_tensor("wqT", [H, FPC], BF16, kind="ExternalInput")
    wkT = nc.dram_tensor("wkT", [H, FPC], BF16, kind="ExternalInput")
    wvT = nc.dram_tensor("wvT", [H, FPC], BF16, kind="ExternalInput")
    woT = nc.dram_tensor("woT", [H, H], BF16, kind="ExternalInput")
    out = nc.dram_tensor("out", [NT // N_CORES, H], F32, kind="ExternalOutput")

    from contextlib import ExitStack

    with tile.TileContext(nc) as tc, ExitStack() as ctx:
        if True:
            pool = lambda **kw: ctx.enter_context(tc.tile_pool(**kw))
            qk_res = pool(name="qk_res", bufs=1)
            v_res = pool(name="v_res", bufs=32)
            o_res = pool(name="o_res", bufs=1)
            const = pool(name="const", bufs=1)
            w_p1 = pool(name="w_p1", bufs=16)
            x_p1 = pool(name="x_p1", bufs=42)
            p_p2 = pool(name="p_p2", bufs=3)
            s_p2 = pool(name="s_p2", bufs=2)
            r_p2 = pool(name="r_p2", bufs=2)
            wo_p3 = pool(name="wo_p3", bufs=32)
            orecv_p = pool(name="orecv_p", bufs=16)
            ev_p3 = pool(name="ev_p3", bufs=3)
            ps_qk = pool(name="ps_qk", bufs=1, space="PSUM")
            ps_sc = pool(name="ps_sc", bufs=1, space="PSUM")
            ps_pv = pool(name="ps_pv", bufs=2, space="PSUM")
            ps_sum = pool(name="ps_sum", bufs=1, space="PSUM")
            dram = pool(name="dram", bufs=1, space="DRAM")
            ones_f = const.tile([128, 128], F32)
            nc.vector.memset(ones_f[:], 1.0)
            ones = const.tile([128, 128], BF16)
            nc.vector.tensor_copy(ones[:], ones_f[:])

            qhat = [qk_res.tile([128, NT], BF16, tag=f"q{m}", name=f"qhat{m}")
                    for m in range(2)]
            khat = [qk_res.tile([128, NT], BF16, tag=f"k{m}", name=f"khat{m}")
                    for m in range(2)]
            vsb = [v_res.tile([128, FPC], BF16, tag="v", name=f"vsb{i}")
                   for i in range(NT // 128)]
            ohat = [o_res.tile([128, NT], BF16, tag=f"o{m}", name=f"ohat{m}")
                    for m in range(2)]
            o_send = dram.tile([N_CORES * FPC, NT // N_CORES], BF16, name="o_send")
            o_recv = dram.tile([N_CORES * FPC, NT // N_CORES], BF16, name="o_recv")

            def load_w(wsrc, tag):
                lst = []
                for kt in range(KT):
                    t = w_p1.tile([128, FPC], BF16, tag=tag, name=f"{tag}{kt}")
                    nc.sync.dma_start(t[:], wsrc[kt * 128 : (kt + 1) * 128, :])
                    lst.append(t)
                return lst

            x_tiles = {}

            def load_x(nb_list):
                for nb in nb_list:
                    for kt in range(KT):
                        t = x_p1.tile([128, NB], BF16, tag="x", name=f"x{nb}_{kt}")
                        nc.sync.dma_start(
                            t[:],
                            xT[kt * 128 : (kt + 1) * 128, nb * NB : (nb + 1) * NB],
                        )
                        x_tiles[nb, kt] = t

            def phase1(b):
                for nb in range(b * S // NB, (b + 1) * S // NB):
                    xt = [x_tiles[nb, kt] for kt in range(KT)]
                    for dest, wt in ((qhat, wq_t), (khat, wk_t)):
                        for m in range(2):
                            ps = ps_qk.tile([128, NB], F32, tag="qk")
                            for kt in range(KT):
                                nc.tensor.matmul(
                                    ps[:],
                                    wt[kt][:, m * 128 : (m + 1) * 128],
                                    xt[kt][:],
                                    start=(kt == 0),
                                    stop=(kt == KT - 1),
                                )
                            nc.vector.tensor_copy(
                                dest[m][:, nb * NB : (nb + 1) * NB], ps[:]
                            )
                    for sub in range(NB // 128):
                        ps = ps_qk.tile([128, FPC], F32, tag="qk")
                        for kt in range(KT):
                            nc.tensor.matmul(
                                ps[:],
                                xt[kt][:, sub * 128 : (sub + 1) * 128],
                                wv_t[kt][:],
                                start=(kt == 0),
                                stop=(kt == KT - 1),
                            )
                        nc.vector.tensor_copy(vsb[nb * 4 + sub][:], ps[:])

            JB = S // 128  # 16 key blocks per (b, h)

            def attention(b, h, qt):
                q_bh = qhat[h][:, b * S : (b + 1) * S]
                k_bh = khat[h][:, b * S : (b + 1) * S]
                pv_ps = ps_pv.tile([128, QT], F32, tag="pv")
                sum_ps = ps_sum.tile([128, QT], F32, tag="sum")
                for g in range(JB // 4):
                    sc_ps = ps_sc.tile([128, EXPW], F32, tag="sc")
                    pT = p_p2.tile([128, EXPW], BF16, tag="p")
                    for i in range(4):
                        jb = g * 4 + i
                        nc.tensor.matmul(
                            sc_ps[:, i * QT : (i + 1) * QT],
                            k_bh[:, jb * 128 : (jb + 1) * 128],
                            q_bh[:, qt * QT : (qt + 1) * QT],
                            start=True,
                            stop=True,
                        )
                    nc.scalar.activation(pT[:], sc_ps[:], Exp, scale=SCALE)
                    for i in range(4):
                        jb = g * 4 + i
                        nc.tensor.matmul(
                            sum_ps[:],
                            ones[:],
                            pT[:, i * QT : (i + 1) * QT],
                            start=(jb == 0),
                            stop=(jb == JB - 1),
                        )
                        nc.tensor.matmul(
                            pv_ps[:],
                            vsb[b * JB + jb][:, h * 128 : (h + 1) * 128],
                            pT[:, i * QT : (i + 1) * QT],
                            start=(jb == 0),
                            stop=(jb == JB - 1),
                        )
                sums = s_p2.tile([128, QT], F32, tag="s")
                nc.vector.tensor_copy(sums[:], sum_ps[:])
                recip = r_p2.tile([128, QT], F32, tag="r")
                nc.vector.reciprocal(recip[:], sums[:])
                nc.vector.tensor_mul(
                    ohat[h][:, b * S + qt * QT : b * S + (qt + 1) * QT],
                    pv_ps[:],
                    recip[:],
                )

            def ship_o(b, qt):
                c = b * (S // QT) + qt  # 512-token send-block index
                for m in range(2):
                    nc.sync.dma_start(
                        o_send[c * FPC + m * 128 : c * FPC + (m + 1) * 128, :],
                        ohat[m][:, b * S + qt * QT : b * S + (qt + 1) * QT],
                    )

            wq_t = []
            for kt in range(KT):
                t = w_p1.tile([128, FPC], BF16, tag="wq", name=f"wq{kt}")
                nc.sync.dma_start(t[:], wqT[kt * 128 : (kt + 1) * 128, :])
                wq_t.append(t)
                tx = x_p1.tile([128, NB], BF16, tag="x", name=f"x0_{kt}")
                nc.sync.dma_start(tx[:], xT[kt * 128 : (kt + 1) * 128, 0:NB])
                x_tiles[0, kt] = tx
            wk_t = load_w(wkT, "wk")
            wv_t = load_w(wvT, "wv")
            load_x([1, 2, 3, 4, 5, 6, 7])
            for b in range(B):
                phase1(b)
                for qt in range(S // QT):
                    for h in range(HPC):
                        attention(b, h, qt)
                    ship_o(b, qt)

            # wo tiles stream on the scalar queue so they transfer
            # during attention instead of queueing behind the A2A-gated
            # o_recv loads on the sync queue.
            wo_tiles = {}
            for nt in range(H // 512):
                for kt in range(KT):
                    t = wo_p3.tile([128, 512], BF16, tag="wo", name=f"wo{nt}_{kt}")
                    nc.scalar.dma_start(
                        t[:], woT[kt * 128 : (kt + 1) * 128, nt * 512 : (nt + 1) * 512]
                    )
                    wo_tiles[nt, kt] = t

            nc.gpsimd.collective_compute(
                "AllToAll",
                mybir.AluOpType.bypass,
                replica_groups=[list(range(N_CORES))],
                ins=[o_send[:].opt()],
                outs=[o_recv[:].opt()],
            )

            # phase 3: out[tok_slice, :] = o_recv.T @ woT  (contraction over H)
            orecv_t = []
            for kt in range(KT):
                t = orecv_p.tile([128, NT // N_CORES], BF16, tag="or", name=f"or{kt}")
                nc.sync.dma_start(t[:], o_recv[kt * 128 : (kt + 1) * 128, :])
                orecv_t.append(t)
            for nt in range(H // 512):
                for tb in range(NT // N_CORES // 128):
                    ps = ps_pv.tile([128, 512], F32, tag="pv")
                    for kt in range(KT):
                        nc.tensor.matmul(
                            ps[:],
                            orecv_t[kt][:, tb * 128 : (tb + 1) * 128],
                            wo_tiles[nt, kt][:],
                            start=(kt == 0),
                            stop=(kt == KT - 1),
                        )
                    ev = ev_p3.tile([128, 512], F32, tag="ev")
                    nc.vector.tensor_copy(ev[:], ps[:])
                    nc.sync.dma_start(
                        out[tb * 128 : (tb + 1) * 128, nt * 512 : (nt + 1) * 512],
                        ev[:],
                    )
    nc.compile()
    return nc


_NC_CACHE = None


def _get_nc():
    global _NC_CACHE
    if _NC_CACHE is None:
        _NC_CACHE = _build()
    return _NC_CACHE


def make_in_maps(x, wq, wk, wv, wo):
    import ml_dtypes

    bf = ml_dtypes.bfloat16
    x = np.asarray(x, dtype=np.float32)
    # tokens b-major: t = b*S + s
    xT = np.ascontiguousarray(x.transpose(2, 1, 0).reshape(H, NT)).astype(bf)
    woT_full = np.ascontiguousarray(np.asarray(wo, dtype=np.float32).T).astype(bf)
    in_maps = []
    for r in range(N_CORES):
        sl = slice(r * FPC, (r + 1) * FPC)
        in_maps.append(
            {
                "xT": xT,
                "wqT": np.ascontiguousarray(np.asarray(wq)[sl, :].T).astype(bf),
                "wkT": np.ascontiguousarray(np.asarray(wk)[sl, :].T).astype(bf),
                "wvT": np.ascontiguousarray(np.asarray(wv)[sl, :].T).astype(bf),
                "woT": woT_full,
            }
        )
    return in_maps


def assemble_out(results):
    out_bs = np.concatenate([results[r]["out"] for r in range(N_CORES)], axis=0)
    return np.ascontiguousarray(out_bs.reshape(B, S, H).transpose(1, 0, 2))


def kernel(x, wq, wk, wv, wo):
    from concourse.bass_utils import run_bass_kernel_spmd

    in_maps = make_in_maps(x, wq, wk, wv, wo)
    res = run_bass_kernel_spmd(_get_nc(), in_maps, list(range(N_CORES)))
    return assemble_out(res.results)


# revision 18
# speedup vs baseline: 1.1243x; 1.1243x over previous
"""Tensor-parallel attention kernel for 8 Trainium2 NeuronCores.

Reference computation (S=2048, B=2, H=2048, NH=16 heads, HD=128):
    q = x @ wq.T ; k = x @ wk.T ; v = x @ wv.T          (x: [S, B, H])
    per (b, head): out = softmax(q k^T / sqrt(HD)) v
    return concat_heads(out) @ wo.T                      ([S, B, H])

Sharding: tensor-parallel over heads. Core r owns heads {2r, 2r+1}:
column-parallel wq/wk/wv shards, row-parallel wo shard, ReduceScatter(add)
over the partial outputs; the host concatenates the 8 scatter slices.

On-core dataflow (bf16 matmuls, f32 accumulation; tokens b-major t = b*S+s):
  phase 1 (per batch): qT/kT [256 feat, 2048 tok] = wT.T @ xT;
                       v [2048 tok, 256 feat] natural
  phase 2 (per b,h,q-tile): scoresT [j, q] blocks -> one wide exp on
      ScalarE -> pT bf16; row sums via ones-matmul (gives broadcast rows);
      PV matmul -> oT [feat, tok]; normalize by 1/sums on VectorE
  phase 3 (per 1024-token chunk): partial[chunk] = oT-slices @ woT ->
      ReduceScatter(add) issued per chunk so it overlaps later compute
"""

import numpy as np

S, B, H = 2048, 2, 2048
NH, HD = 16, 128
N_CORES = 8
HPC = NH // N_CORES          # heads per core (2)
FPC = HPC * HD               # features per core (256)
NT = S * B                   # tokens (4096)
SCALE = HD ** -0.5
KT = H // 128                # contraction tiles in phase 1 (16)
NB = 512                     # token block width in phase 1
QT = 512                     # q-tile width in phase 2
EXPW = 1024                  # exp batch width (2 key-blocks per ACT op)
RS_CHUNKS = 4
CHUNK = NT // RS_CHUNKS      # tokens per chunk (1024)
SLICE = CHUNK // N_CORES     # rows a core receives per chunk (128)


def _build():
    import concourse.mybir as mybir
    import concourse.tile as tile
    from concourse import bacc

    F32 = mybir.dt.float32
    BF16 = mybir.dt.bfloat16
    Exp = mybir.ActivationFunctionType.Exp

    nc = bacc.Bacc(None, target_bir_lowering=False, num_devices=N_CORES)

    xT = nc.dram_tensor("xT", [H, NT], BF16, kind="ExternalInput")
    wqT = nc.dram_tensor("wqT", [H, FPC], BF16, kind="ExternalInput")
    wkT = nc.dram_tensor("wkT", [H, FPC], BF16, kind="ExternalInput")
    wvT = nc.dram_tensor("wvT", [H, FPC], BF16, kind="ExternalInput")
    woT = nc.dram_tensor("woT", [H, H], BF16, kind="ExternalInput")
    out = nc.dram_tensor("out", [NT // N_CORES, H], F32, kind="ExternalOutput")

    from contextlib import ExitStack

    with tile.TileContext(nc) as tc, ExitStack() as ctx:
        if True:
            pool = lambda **kw: ctx.enter_context(tc.tile_pool(**kw))
            qk_res = pool(name="qk_res", bufs=1)
            v_res = pool(name="v_res", bufs=32)
            o_res = pool(name="o_res", bufs=1)
            const = pool(name="const", bufs=1)
            w_p1 = pool(name="w_p1", bufs=16)
            x_p1 = pool(name="x_p1", bufs=42)
            p_p2 = pool(name="p_p2", bufs=6)
            s_p2 = pool(name="s_p2", bufs=2)
            r_p2 = pool(name="r_p2", bufs=2)
            wo_p3 = pool(name="wo_p3", bufs=32)
            orecv_p = pool(name="orecv_p", bufs=16)
            ev_p3 = pool(name="ev_p3", bufs=3)
            ps_qk = pool(name="ps_qk", bufs=1, space="PSUM")
            ps_sc = pool(name="ps_sc", bufs=2, space="PSUM")
            ps_pv = pool(name="ps_pv", bufs=2, space="PSUM")
            ps_sum = pool(name="ps_sum", bufs=1, space="PSUM")
            dram = pool(name="dram", bufs=1, space="DRAM")
            ones_f = const.tile([128, 128], F32)
            nc.vector.memset(ones_f[:], 1.0)
            ones = const.tile([128, 128], BF16)
            nc.vector.tensor_copy(ones[:], ones_f[:])

            qhat = [qk_res.tile([128, NT], BF16, tag=f"q{m}", name=f"qhat{m}")
                    for m in range(2)]
            khat = [qk_res.tile([128, NT], BF16, tag=f"k{m}", name=f"khat{m}")
                    for m in range(2)]
            vsb = [v_res.tile([128, FPC], BF16, tag="v", name=f"vsb{i}")
                   for i in range(NT // 128)]
            ohat = [o_res.tile([128, NT], BF16, tag=f"o{m}", name=f"ohat{m}")
                    for m in range(2)]
            o_send = dram.tile([N_CORES * FPC, NT // N_CORES], BF16, name="o_send")
            o_recv = dram.tile([N_CORES * FPC, NT // N_CORES], BF16, name="o_recv")

            def load_w(wsrc, tag):
                lst = []
                for kt in range(KT):
                    t = w_p1.tile([128, FPC], BF16, tag=tag, name=f"{tag}{kt}")
                    nc.sync.dma_start(t[:], wsrc[kt * 128 : (kt + 1) * 128, :])
                    lst.append(t)
                return lst

            x_tiles = {}

            def load_x(nb_list):
                for nb in nb_list:
                    for kt in range(KT):
                        t = x_p1.tile([128, NB], BF16, tag="x", name=f"x{nb}_{kt}")
                        nc.sync.dma_start(
                            t[:],
                            xT[kt * 128 : (kt + 1) * 128, nb * NB : (nb + 1) * NB],
                        )
                        x_tiles[nb, kt] = t

            def phase1(b):
                for nb in range(b * S // NB, (b + 1) * S // NB):
                    xt = [x_tiles[nb, kt] for kt in range(KT)]
                    for dest, wt in ((qhat, wq_t), (khat, wk_t)):
                        for m in range(2):
                            ps = ps_qk.tile([128, NB], F32, tag="qk")
                            for kt in range(KT):
                                nc.tensor.matmul(
                                    ps[:],
                                    wt[kt][:, m * 128 : (m + 1) * 128],
                                    xt[kt][:],
                                    start=(kt == 0),
                                    stop=(kt == KT - 1),
                                )
                            nc.vector.tensor_copy(
                                dest[m][:, nb * NB : (nb + 1) * NB], ps[:]
                            )
                    for sub in range(NB // 128):
                        ps = ps_qk.tile([128, FPC], F32, tag="qk")
                        for kt in range(KT):
                            nc.tensor.matmul(
                                ps[:],
                                xt[kt][:, sub * 128 : (sub + 1) * 128],
                                wv_t[kt][:],
                                start=(kt == 0),
                                stop=(kt == KT - 1),
                            )
                        nc.vector.tensor_copy(vsb[nb * 4 + sub][:], ps[:])

            JB = S // 128  # 16 key blocks per (b, h)

            def attention(b, h, qt):
                q_bh = qhat[h][:, b * S : (b + 1) * S]
                k_bh = khat[h][:, b * S : (b + 1) * S]
                pv_ps = ps_pv.tile([128, QT], F32, tag="pv")
                sum_ps = ps_sum.tile([128, QT], F32, tag="sum")
                for g in range(JB // 2):
                    sc_ps = ps_sc.tile([128, EXPW], F32, tag="sc")
                    pT = p_p2.tile([128, EXPW], BF16, tag="p")
                    for i in range(2):
                        jb = g * 2 + i
                        nc.tensor.matmul(
                            sc_ps[:, i * QT : (i + 1) * QT],
                            k_bh[:, jb * 128 : (jb + 1) * 128],
                            q_bh[:, qt * QT : (qt + 1) * QT],
                            start=True,
                            stop=True,
                        )
                    nc.scalar.activation(pT[:], sc_ps[:], Exp, scale=SCALE)
                    for i in range(2):
                        jb = g * 2 + i
                        nc.tensor.matmul(
                            sum_ps[:],
                            ones[:],
                            pT[:, i * QT : (i + 1) * QT],
                            start=(jb == 0),
                            stop=(jb == JB - 1),
                        )
                        nc.tensor.matmul(
                            pv_ps[:],
                            vsb[b * JB + jb][:, h * 128 : (h + 1) * 128],
                            pT[:, i * QT : (i + 1) * QT],
                            start=(jb == 0),
                            stop=(jb == JB - 1),
                        )
                sums = s_p2.tile([128, QT], F32, tag="s")
                nc.vector.tensor_copy(sums[:], sum_ps[:])
                recip = r_p2.tile([128, QT], F32, tag="r")
                nc.vector.reciprocal(recip[:], sums[:])
                nc.vector.tensor_mul(
                    ohat[h][:, b * S + qt * QT : b * S + (qt + 1) * QT],
                    pv_ps[:],
                    recip[:],
                )

            def ship_o(b, qt):
                c = b * (S // QT) + qt  # 512-token send-block index
                for m in range(2):
                    nc.sync.dma_start(
                        o_send[c * FPC + m * 128 : c * FPC + (m + 1) * 128, :],
                        ohat[m][:, b * S + qt * QT : b * S + (qt + 1) * QT],
                    )

            wq_t = []
            for kt in range(KT):
                t = w_p1.tile([128, FPC], BF16, tag="wq", name=f"wq{kt}")
                nc.sync.dma_start(t[:], wqT[kt * 128 : (kt + 1) * 128, :])
                wq_t.append(t)
                tx = x_p1.tile([128, NB], BF16, tag="x", name=f"x0_{kt}")
                nc.sync.dma_start(tx[:], xT[kt * 128 : (kt + 1) * 128, 0:NB])
                x_tiles[0, kt] = tx
            wk_t = load_w(wkT, "wk")
            wv_t = load_w(wvT, "wv")
            load_x([1, 2, 3, 4, 5, 6, 7])
            for b in range(B):
                phase1(b)
                for qt in range(S // QT):
                    for h in range(HPC):
                        attention(b, h, qt)
                    ship_o(b, qt)

            # wo tiles stream on the scalar queue so they transfer
            # during attention instead of queueing behind the A2A-gated
            # o_recv loads on the sync queue.
            wo_tiles = {}
            for nt in range(H // 512):
                for kt in range(KT):
                    t = wo_p3.tile([128, 512], BF16, tag="wo", name=f"wo{nt}_{kt}")
                    nc.scalar.dma_start(
                        t[:], woT[kt * 128 : (kt + 1) * 128, nt * 512 : (nt + 1) * 512]
                    )
                    wo_tiles[nt, kt] = t

            nc.gpsimd.collective_compute(
                "AllToAll",
                mybir.AluOpType.bypass,
                replica_groups=[list(range(N_CORES))],
                ins=[o_send[:].opt()],
                outs=[o_recv[:].opt()],
            )

            # phase 3: out[tok_slice, :] = o_recv.T @ woT  (contraction over H)
            orecv_t = []
            for kt in range(KT):
                t = orecv_p.tile([128, NT // N_CORES], BF16, tag="or", name=f"or{kt}")
                nc.sync.dma_start(t[:], o_recv[kt * 128 : (kt + 1) * 128, :])
                orecv_t.append(t)
            for nt in range(H // 512):
                for tb in range(NT // N_CORES // 128):
                    ps = ps_pv.tile([128, 512], F32, tag="pv")
                    for kt in range(KT):
                        nc.tensor.matmul(
                            ps[:],
                            orecv_t[kt][:, tb * 128 : (tb + 1) * 128],
                            wo_tiles[nt, kt][:],
                            start=(kt == 0),
                            stop=(kt == KT - 1),
                        )
                    ev = ev_p3.tile([128, 512], F32, tag="ev")
                    nc.vector.tensor_copy(ev[:], ps[:])
                    nc.sync.dma_start(
                        out[tb * 128 : (tb + 1) * 128, nt * 512 : (nt + 1) * 512],
                        ev[:],
                    )
    nc.compile()
    return nc


_NC_CACHE = None


def _get_nc():
    global _NC_CACHE
    if _NC_CACHE is None:
        _NC_CACHE = _build()
    return _NC_CACHE


def make_in_maps(x, wq, wk, wv, wo):
    import ml_dtypes

    bf = ml_dtypes.bfloat16
    x = np.asarray(x, dtype=np.float32)
    # tokens b-major: t = b*S + s
    xT = np.ascontiguousarray(x.transpose(2, 1, 0).reshape(H, NT)).astype(bf)
    woT_full = np.ascontiguousarray(np.asarray(wo, dtype=np.float32).T).astype(bf)
    in_maps = []
    for r in range(N_CORES):
        sl = slice(r * FPC, (r + 1) * FPC)
        in_maps.append(
            {
                "xT": xT,
                "wqT": np.ascontiguousarray(np.asarray(wq)[sl, :].T).astype(bf),
                "wkT": np.ascontiguousarray(np.asarray(wk)[sl, :].T).astype(bf),
                "wvT": np.ascontiguousarray(np.asarray(wv)[sl, :].T).astype(bf),
                "woT": woT_full,
            }
        )
    return in_maps


def assemble_out(results):
    out_bs = np.concatenate([results[r]["out"] for r in range(N_CORES)], axis=0)
    return np.ascontiguousarray(out_bs.reshape(B, S, H).transpose(1, 0, 2))


def kernel(x, wq, wk, wv, wo):
    from concourse.bass_utils import run_bass_kernel_spmd

    in_maps = make_in_maps(x, wq, wk, wv, wo)
    res = run_bass_kernel_spmd(_get_nc(), in_maps, list(range(N_CORES)))
    return assemble_out(res.results)
